# revision 44
# baseline (speedup 1.0000x reference)
"""Trainium2 Bass kernel for nn_AttentionBlock (GroupNorm + MHA + proj + residual).

Sharding: data-parallel over batch across 8 NeuronCores (2 images per core).
Per-core pipeline (all fp32, matmuls in float32r for full PE rate):
  - GroupNorm(32) stats via bn_stats + tiny PE partition-reductions
  - qkv 1x1 conv as channel GEMMs with GN affine + attention scale folded in
  - attention per head: S^T = k^T q (s on partitions), max-free softmax exp on
    ACT, PV matmul whose lhsT carries 64 "ones" columns so the softmax
    denominator lands replicated on the other 64 output partitions
  - proj GEMM + residual add
"""

import numpy as np

B, C, HW, T = 16, 512, 32, 1024
G = 32                      # groups
NH = 8                      # heads
CH = C // NH                # 64 head dim
NCORES = 8
BL = B // NCORES            # images per core
KT = C // 128               # 4 channel tiles
EPS = 1e-5

_CACHE = {}


def _build_nc():
    import concourse.bacc as bacc
    import concourse.tile as tile
    from concourse import mybir
    from concourse.tile import add_dep_helper

    F32 = mybir.dt.float32
    F32R = mybir.dt.float32r
    AF = mybir.ActivationFunctionType
    OP = mybir.AluOpType

    nc = bacc.Bacc()
    x_d = nc.declare_dram_parameter("x", [BL, C, T], F32, isOutput=False)
    wq_d = nc.declare_dram_parameter("wqT", [C, C], F32R, isOutput=False)
    wk_d = nc.declare_dram_parameter("wkT", [C, C], F32R, isOutput=False)
    wv_d = nc.declare_dram_parameter("wvT", [C, C], F32R, isOutput=False)
    wp_d = nc.declare_dram_parameter("projT", [C, C], F32R, isOutput=False)
    r_d = nc.declare_dram_parameter("Rm", [128, G], F32, isOutput=False)
    b_d = nc.declare_dram_parameter("Bm", [G, 128], F32, isOutput=False)
    bq_d = nc.declare_dram_parameter("bq", [128, KT], F32, isOutput=False)
    bk_d = nc.declare_dram_parameter("bk", [128, KT], F32, isOutput=False)
    ob_d = nc.declare_dram_parameter("ob", [128, KT], F32, isOutput=False)
    ones_d = nc.declare_dram_parameter("ones", [128, 4, CH], F32R, isOutput=False)
    y_d = nc.declare_dram_parameter("y", [BL, C, T], F32, isOutput=True)

    def mm(out, lhsT, rhs, start, stop):
        nc.tensor.matmul(out, lhsT, rhs, start=start, stop=stop)

    with tile.TileContext(nc) as tc:
        with (
            tc.tile_pool(name="const", bufs=1) as cp,
            tc.tile_pool(name="data", bufs=1) as dp,
            tc.tile_pool(name="xbuf", bufs=2) as xp,
            tc.tile_pool(name="work", bufs=2) as wp,
            tc.tile_pool(name="small", bufs=2) as sp,
            tc.tile_pool(name="ps", bufs=2, space="PSUM") as ps,
            tc.tile_pool(name="ps1", bufs=2, space="PSUM") as ps1,
            tc.tile_pool(name="pp", bufs=2, space="PSUM") as pp,
        ):
            # ---- image-0 x load first: it heads the critical path ----
            x_ts = [xp.tile([128, KT, T], F32, tag="x", name=f"x{i}") for i in range(BL)]
            for kt in range(KT):
                nc.sync.dma_start(out=x_ts[0][:, kt, :], in_=x_d[0, kt * 128:(kt + 1) * 128, :])

            # ---- x1 immediately after x0 on SP (keeps the hoisted img1
            # bn_stats from stalling the DVE FIFO); tiny consts first on
            # gpsimd, then weights in first-use order ----
            gated_insts = []
            for img in range(1, BL):
                for kt in range(KT):
                    di = nc.sync.dma_start(out=x_ts[img][:, kt, :],
                                           in_=x_d[img, kt * 128:(kt + 1) * 128, :])
                    gated_insts.append(di)
            ones_sb = cp.tile([128, 4, CH], F32R, tag="onesb")
            nc.sync.dma_start(out=ones_sb, in_=ones_d[:, :, :])
            r_sb = cp.tile([128, G], F32, tag="rm")
            nc.gpsimd.dma_start(out=r_sb, in_=r_d[:, :])
            bm_sb = cp.tile([G, 128], F32, tag="bm")
            nc.gpsimd.dma_start(out=bm_sb, in_=b_d[:, :])
            bq_sb = cp.tile([128, KT], F32, tag="bq")
            nc.gpsimd.dma_start(out=bq_sb, in_=bq_d[:, :])
            bk_sb = cp.tile([128, KT], F32, tag="bk")
            nc.gpsimd.dma_start(out=bk_sb, in_=bk_d[:, :])
            ob_sb = cp.tile([128, KT], F32, tag="ob")
            nc.gpsimd.dma_start(out=ob_sb, in_=ob_d[:, :])
            wq_sb = cp.tile([128, KT, C], F32R, tag="wq")
            wk_sb = cp.tile([128, KT, C], F32R, tag="wk")
            wv_sb = cp.tile([128, KT, C], F32R, tag="wv")
            wp_sb = cp.tile([128, KT, C], F32R, tag="wp")
            for w_sb, w_dr in ((wk_sb, wk_d), (wq_sb, wq_d), (wv_sb, wv_d), (wp_sb, wp_d)):
                nc.gpsimd.dma_start(
                    out=w_sb, in_=w_dr.rearrange("(kt p) m -> p kt m", p=128))

            # vT tiles: per s-block [t(128part), 4 pairs x (v_even|ones|v_odd)];
            # ones columns replicated from ones_sb by cheap DVE copies
            vt_all = [[dp.tile([128, 4, 3, CH], F32R, tag=f"vt{s}", bufs=2,
                                name=f"vt{img}_{s}") for s in range(8)]
                      for img in range(BL)]

            xn_ts = [None] * BL
            q_ts = [None] * BL
            k_ts = [None] * BL
            a_ts = [None] * BL

            def front(img):
                """x DMA (img>0), per-kt GroupNorm stats -> xn (each 128-ch
                tile holds exactly 8 whole groups, so the chain pipelines)."""
                x_t = x_ts[img]
                xn_t = dp.tile([128, KT, T], F32R, tag="xn", name=f"xn{img}")
                sts = []
                for kt in range(KT):
                    bn = sp.tile([128, 2, 6], F32, tag="bn")
                    nc.vector.bn_stats(out=bn[:, 0, :], in_=x_t[:, kt, 0:512])
                    nc.vector.bn_stats(out=bn[:, 1, :], in_=x_t[:, kt, 512:1024])
                    mv = sp.tile([128, 2], F32, tag="mv")
                    nc.vector.bn_aggr(out=mv, in_=bn)
                    st = sp.tile([128, 2], F32, tag=f"st{kt}", name=f"st{img}_{kt}")
                    nc.vector.tensor_copy(out=st[:, 0:1], in_=mv[:, 0:1])
                    nc.vector.tensor_mul(out=st[:, 1:2], in0=mv[:, 0:1], in1=mv[:, 0:1])
                    nc.vector.tensor_add(out=st[:, 1:2], in0=st[:, 1:2], in1=mv[:, 1:2])
                    sts.append(st)
                gsum = pp.tile([8, 2 * KT], F32, tag="pm", name=f"gsum{img}")
                for kt in range(KT):
                    nc.tensor.matmul(gsum[:, 2 * kt:2 * kt + 2], r_sb[:, 0:8], sts[kt],
                                     start=True, stop=True)
                gs_sb = sp.tile([8, 2 * KT], F32, tag="gs")
                nc.vector.tensor_copy(out=gs_sb, in_=gsum)
                gstat = sp.tile([8, 2 * KT], F32, tag="gstat")  # [rstd, beta] per kt
                var = sp.tile([8, KT], F32, tag="gvar")
                tmp = sp.tile([8, KT], F32, tag="gtmp")
                y = sp.tile([8, KT], F32, tag="gy")
                ge = gs_sb[:, 0:2 * KT:2]
                go = gs_sb[:, 1:2 * KT:2]
                nc.vector.tensor_mul(out=tmp, in0=ge, in1=ge)
                nc.vector.tensor_sub(out=var, in0=go, in1=tmp)
                nc.vector.tensor_scalar_add(out=var, in0=var, scalar1=EPS)
                # rsqrt via bit-trick seed + 3 Newton steps (all DVE; the
                # magic-constant subtraction runs in fp32 value domain since
                # DVE integer add is routed through fp32)
                u32 = mybir.dt.uint32
                yu = sp.tile([8, KT], u32, tag="gyu")
                nc.vector.tensor_scalar(out=yu, in0=var.bitcast(u32), scalar1=1,
                                        scalar2=None, op0=OP.logical_shift_right)
                nc.vector.tensor_copy(out=tmp, in_=yu)
                nc.vector.tensor_scalar(out=tmp, in0=tmp,
                                        scalar1=float(0x5f3759df), scalar2=-1.0,
                                        op0=OP.subtract, op1=OP.mult)
                nc.vector.tensor_copy(out=y.bitcast(u32), in_=tmp)
                for _ in range(3):
                    nc.vector.tensor_mul(out=tmp, in0=y, in1=y)
                    nc.vector.tensor_mul(out=tmp, in0=tmp, in1=var)
                    nc.vector.tensor_scalar(out=tmp, in0=tmp, scalar1=-0.5,
                                            scalar2=1.5, op0=OP.mult, op1=OP.add)
                    nc.vector.tensor_mul(out=y, in0=y, in1=tmp)
                nc.vector.tensor_copy(out=gstat[:, 0:2 * KT:2], in_=y)
                # beta = -mean * rstd
                nc.vector.tensor_mul(out=tmp, in0=ge, in1=y)
                nc.vector.tensor_scalar_mul(out=gstat[:, 1:2 * KT:2], in0=tmp,
                                            scalar1=-1.0)
                cstat = pp.tile([128, 2 * KT], F32, tag="pm", name=f"cstat{img}")
                for kt in range(KT):
                    nc.tensor.matmul(cstat[:, 2 * kt:2 * kt + 2], bm_sb[0:8, :],
                                     gstat[:, 2 * kt:2 * kt + 2],
                                     start=True, stop=True)
                cs_sb = sp.tile([128, 2 * KT], F32, tag="cs")
                nc.vector.tensor_copy(out=cs_sb, in_=cstat)
                for kt in range(KT):
                    nc.vector.tensor_scalar(out=xn_t[:, kt, :], in0=x_t[:, kt, :],
                                            scalar1=cs_sb[:, 2 * kt:2 * kt + 1],
                                            scalar2=cs_sb[:, 2 * kt + 1:2 * kt + 2],
                                            op0=OP.mult, op1=OP.add)
                for sblk in range(8):
                    ci = nc.vector.tensor_copy(out=vt_all[img][sblk][:, :, 1, :],
                                               in_=ones_sb)
                    if img > 0:
                        gated_insts.append(ci)
                xn_ts[img] = xn_t
                q_ts[img] = dp.tile([128, KT, T], F32R, tag="q", name=f"q{img}")
                k_ts[img] = dp.tile([128, KT, T], F32R, tag="k", name=f"k{img}")
                a_ts[img] = dp.tile([128, KT, T], F32R, tag="a", name=f"a{img}")

            def emit_v(img, s):
                pm = pp.tile([128, 512], F32, tag="pm", name=f"v{img}_{s}")
                for kt in range(KT):
                    mm(pm, xn_ts[img][:, kt, s * 128:(s + 1) * 128], wv_sb[:, kt, :],
                       kt == 0, kt == KT - 1)
                nc.vector.tensor_copy(
                    out=vt_all[img][s][:, :, 0:3:2, :],
                    in_=pm.rearrange("p (pr t c) -> p pr t c", pr=4, t=2))

            def emit_qk_group(img, which, mt, chk):
                w_sb, b_sb = (wq_sb, bq_sb) if which == "q" else (wk_sb, bk_sb)
                dst = q_ts[img] if which == "q" else k_ts[img]
                pm = pp.tile([128, 512], F32, tag="pm", name=f"qk{img}_{which}{mt}{chk}")
                for kt in range(KT):
                    mm(pm, w_sb[:, kt, mt * 128:(mt + 1) * 128],
                       xn_ts[img][:, kt, chk * 512:(chk + 1) * 512],
                       kt == 0, kt == KT - 1)
                nc.vector.tensor_scalar_add(
                    out=dst[:, mt, chk * 512:(chk + 1) * 512], in0=pm,
                    scalar1=b_sb[:, mt:mt + 1])

            def emit_proj(img, mt, chk, pool=None, ptag="pm"):
                pool = pool or pp
                pm = pool.tile([128, 512], F32, tag=ptag, name=f"pj{img}_{mt}{chk}")
                for kt in range(KT):
                    mm(pm, wp_sb[:, kt, mt * 128:(mt + 1) * 128],
                       a_ts[img][:, kt, chk * 512:(chk + 1) * 512],
                       kt == 0, kt == KT - 1)
                o_t = wp.tile([128, 512], F32, tag="o", bufs=4)
                nc.vector.scalar_tensor_tensor(
                    out=o_t, in0=pm, scalar=ob_sb[:, mt:mt + 1],
                    in1=x_ts[img][:, mt, chk * 512:(chk + 1) * 512],
                    op0=OP.add, op1=OP.add)
                eng = nc.sync if (mt + chk) % 2 == 0 else nc.gpsimd
                eng.dma_start(
                    out=y_d[img, mt * 128:(mt + 1) * 128, chk * 512:(chk + 1) * 512],
                    in_=o_t)

            # ---- prolog: image 0 front-end + v(s<2) + qk(mt=0, chk=0) ----
            front(0)
            emit_qk_group(0, "k", 0, 0)
            emit_qk_group(0, "k", 0, 1)
            emit_qk_group(0, "q", 0, 0)

            # ---- pending-work schedule keyed by global unit index ----
            sched = {}

            def put(u, fn):
                sched.setdefault(u, []).append(fn)

            for s in range(2, 8):
                put(s - 1, lambda s=s: emit_v(0, s))
            put(5, lambda: emit_qk_group(0, "q", 0, 1))
            # per-mt order: k chk0, q chk0, k chk1, q chk1
            qk4 = [("k", 0), ("q", 0), ("k", 1), ("q", 1)]
            for j, (w, chk) in enumerate(qk4):
                put(9 + j, lambda w=w, chk=chk: emit_qk_group(0, w, 1, chk))
                put(25 + j, lambda w=w, chk=chk: emit_qk_group(0, w, 2, chk))
                put(37 + j, lambda w=w, chk=chk: emit_qk_group(0, w, 3, chk))
            if BL > 1:
                put(44, lambda: front(1))
                for j, (w, chk) in enumerate(qk4):
                    put(56 + j, lambda w=w, chk=chk: emit_qk_group(1, w, 0, chk))
                    put(73 + j, lambda w=w, chk=chk: emit_qk_group(1, w, 1, chk))
                    put(89 + j, lambda w=w, chk=chk: emit_qk_group(1, w, 2, chk))
                    put(105 + j, lambda w=w, chk=chk: emit_qk_group(1, w, 3, chk))
                for s in range(8):
                    put(48 + s, lambda s=s: emit_v(1, s))
                for j in range(8):
                    put(65 + j, lambda j=j: emit_proj(0, j // 2, j % 2))

            # ---- attention: flat pipeline over (img, pair, t-half, s) units.
            # The two heads of a pair run as concurrent row-tiled QK matmuls
            # (lhsT partitions 0-63 vs 64-127); one exp covers both heads'
            # t-half chunk; PV accumulates into per-head 1-bank psum halves.
            units = [(img, pr, th, s) for img in range(BL)
                     for pr in range(4) for th in range(2) for s in range(8)]

            def emit_qkmm(img, pr, th, s):
                S = ps.tile([128, T], F32, tag="ps", name=f"S{img}_{pr}_{th}_{s}")
                for odd in range(2):
                    mm(S[:, odd * 512:(odd + 1) * 512],
                       k_ts[img][odd * 64:(odd + 1) * 64, pr, s * 128:(s + 1) * 128],
                       q_ts[img][odd * 64:(odd + 1) * 64, pr, th * 512:(th + 1) * 512],
                       True, True)
                return S

            gate_box = []
            pva = pvb = None
            p_prev = None
            S_cur = emit_qkmm(*units[0])
            for s_ in range(2):
                emit_v(0, s_)
            for i, (img, pr, th, s) in enumerate(units):
                p_t = wp.tile([128, T], F32R, tag="p", bufs=4)
                exp_inst = nc.scalar.activation(out=p_t, in_=S_cur, func=AF.Exp)
                if i == 2:
                    gate_box.append(exp_inst)
                if i + 1 < len(units):
                    S_cur = emit_qkmm(*units[i + 1])
                vflat = vt_all[img][s].rearrange("p pr t c -> p (pr t c)")
                if s == 0:
                    # defer PV(s=0) one unit: the previous pair's normalize
                    # still holds the pv slots
                    p_prev = p_t
                else:
                    if s == 1:
                        pva = ps1.tile([128, 512], F32, tag="pv",
                                       name=f"pva{img}_{pr}_{th}")
                        pvb = ps1.tile([128, 512], F32, tag="pv",
                                       name=f"pvb{img}_{pr}_{th}")
                        vflat0 = vt_all[img][0].rearrange("p pr t c -> p (pr t c)")
                        mm(pva, vflat0[:, pr * 192: pr * 192 + 128],
                           p_prev[:, 0:512], True, False)
                        mm(pvb, vflat0[:, pr * 192 + 64: pr * 192 + 192],
                           p_prev[:, 512:1024], True, False)
                    mm(pva, vflat[:, pr * 192: pr * 192 + 128],
                       p_t[:, 0:512], False, s == 7)
                    mm(pvb, vflat[:, pr * 192 + 64: pr * 192 + 192],
                       p_t[:, 512:1024], False, s == 7)
                for fn in sched.pop(i, ()):
                    fn()
                if s == 7:
                    rb = wp.tile([128, 512], F32, tag="rb")
                    tch = slice(th * 512, (th + 1) * 512)
                    nc.vector.reciprocal(out=rb[0:64, :], in_=pva[64:128, :])
                    nc.vector.tensor_mul(out=a_ts[img][0:64, pr, tch],
                                         in0=pva[0:64, :], in1=rb[0:64, :])
                    nc.vector.reciprocal(out=rb[64:128, :], in_=pvb[0:64, :])
                    nc.vector.tensor_mul(out=a_ts[img][64:128, pr, tch],
                                         in0=pvb[64:128, :], in1=rb[64:128, :])

            for gi in gated_insts:
                add_dep_helper(gi.ins, gate_box[0].ins, sync=False,
                               reason="hold img1 prefetch until attention starts")

            # ---- tail: proj of the last image, fanned across psum pools ----
            for j in range(8):
                mt, chk = j // 2, j % 2
                if j % 3 == 0:
                    emit_proj(BL - 1, mt, chk)
                elif j % 3 == 1:
                    emit_proj(BL - 1, mt, chk, pool=ps, ptag="ps")
                else:
                    emit_proj(BL - 1, mt, chk, pool=ps1, ptag="pv")
    nc.finalize()
    return nc


def get_nc():
    if "nc" not in _CACHE:
        _CACHE["nc"] = _build_nc()
    return _CACHE["nc"]


def prep_inputs(x, gn_w, gn_b, qkv_w, qkv_b, proj_w, proj_b):
    """Host-side preprocessing -> list of per-core input maps."""
    f = np.float32
    x = np.asarray(x, f)
    gn_w, gn_b = np.asarray(gn_w, f), np.asarray(gn_b, f)
    qkv_w, qkv_b = np.asarray(qkv_w, f), np.asarray(qkv_b, f)
    proj_w, proj_b = np.asarray(proj_w, f), np.asarray(proj_b, f)

    scale = 1.0 / np.sqrt(np.sqrt(CH))
    W3 = qkv_w.reshape(NH, 3 * CH, C)
    b3 = qkv_b.reshape(NH, 3 * CH)
    Wq = W3[:, 0:CH].reshape(C, C)
    Wk = W3[:, CH:2 * CH].reshape(C, C)
    Wv = W3[:, 2 * CH:3 * CH].reshape(C, C)
    bq = b3[:, 0:CH].reshape(C)
    bk = b3[:, CH:2 * CH].reshape(C)
    bv = b3[:, 2 * CH:3 * CH].reshape(C)

    wqT = ((Wq * gn_w[None, :]) * scale).T.copy()
    wkT = ((Wk * gn_w[None, :]) * scale).T.copy()
    wvT = (Wv * gn_w[None, :]).T.copy()
    bq_e = (Wq @ gn_b + bq) * scale
    bk_e = (Wk @ gn_b + bk) * scale
    bv_e = Wv @ gn_b + bv
    projT = proj_w.T.copy()
    ob = proj_w @ bv_e + proj_b

    gs = C // G  # 16 channels per group; 8 whole groups per 128-channel tile
    Rm = np.zeros((128, G), f)
    Rm[np.arange(128), np.arange(128) // gs] = 1.0 / gs
    Bm = np.zeros((G, 128), f)
    Bm[np.arange(128) // gs, np.arange(128)] = 1.0

    def pack_bias(b):
        return np.ascontiguousarray(b.reshape(KT, 128).T)

    x5 = x.reshape(B, C, T)
    shared = dict(wqT=wqT, wkT=wkT, wvT=wvT, projT=projT, Rm=Rm, Bm=Bm,
                  bq=pack_bias(bq_e), bk=pack_bias(bk_e), ob=pack_bias(ob),
                  ones=np.ones((128, 4, CH), f))
    return [dict(x=np.ascontiguousarray(x5[c * BL:(c + 1) * BL]), **shared)
            for c in range(NCORES)]


def kernel(x, gn_w, gn_b, qkv_w, qkv_b, proj_w, proj_b):
    from concourse.bass_utils import run_bass_kernel_spmd

    in_maps = prep_inputs(x, gn_w, gn_b, qkv_w, qkv_b, proj_w, proj_b)
    nc = get_nc()
    res = run_bass_kernel_spmd(nc, in_maps, list(range(NCORES)))
    out = np.concatenate([res.results[c]["y"] for c in range(NCORES)], axis=0)
    return out.reshape(B, C, HW, HW).astype(np.float32)


# revision 46
# speedup vs baseline: 18778.2160x; 18778.2160x over previous
"""Trainium2 Bass kernel for nn_AttentionBlock (GroupNorm + MHA + proj + residual).

Sharding: data-parallel over batch across 8 NeuronCores (2 images per core).
Per-core pipeline (all fp32, matmuls in float32r for full PE rate):
  - GroupNorm(32) stats via bn_stats + tiny PE partition-reductions
  - qkv 1x1 conv as channel GEMMs with GN affine + attention scale folded in
  - attention per head: S^T = k^T q (s on partitions), max-free softmax exp on
    ACT, PV matmul whose lhsT carries 64 "ones" columns so the softmax
    denominator lands replicated on the other 64 output partitions
  - proj GEMM + residual add
"""

import time

import numpy as np

B, C, HW, T = 16, 512, 32, 1024
G = 32                      # groups
NH = 8                      # heads
CH = C // NH                # 64 head dim
NCORES = 8
BL = B // NCORES            # images per core
KT = C // 128               # 4 channel tiles
EPS = 1e-5

_CACHE = {}


def _build_nc():
    import concourse.bacc as bacc
    import concourse.tile as tile
    from concourse import mybir
    from concourse.tile import add_dep_helper

    F32 = mybir.dt.float32
    F32R = mybir.dt.float32r
    AF = mybir.ActivationFunctionType
    OP = mybir.AluOpType

    nc = bacc.Bacc()
    x_d = nc.declare_dram_parameter("x", [BL, C, T], F32, isOutput=False)
    wq_d = nc.declare_dram_parameter("wqT", [C, C], F32R, isOutput=False)
    wk_d = nc.declare_dram_parameter("wkT", [C, C], F32R, isOutput=False)
    wv_d = nc.declare_dram_parameter("wvT", [C, C], F32R, isOutput=False)
    wp_d = nc.declare_dram_parameter("projT", [C, C], F32R, isOutput=False)
    r_d = nc.declare_dram_parameter("Rm", [128, G], F32, isOutput=False)
    b_d = nc.declare_dram_parameter("Bm", [G, 128], F32, isOutput=False)
    bq_d = nc.declare_dram_parameter("bq", [128, KT], F32, isOutput=False)
    bk_d = nc.declare_dram_parameter("bk", [128, KT], F32, isOutput=False)
    ob_d = nc.declare_dram_parameter("ob", [128, KT], F32, isOutput=False)
    ones_d = nc.declare_dram_parameter("ones", [128, 4, CH], F32R, isOutput=False)
    y_d = nc.declare_dram_parameter("y", [BL, C, T], F32, isOutput=True)

    def mm(out, lhsT, rhs, start, stop):
        nc.tensor.matmul(out, lhsT, rhs, start=start, stop=stop)

    with tile.TileContext(nc) as tc:
        with (
            tc.tile_pool(name="const", bufs=1) as cp,
            tc.tile_pool(name="data", bufs=1) as dp,
            tc.tile_pool(name="xbuf", bufs=2) as xp,
            tc.tile_pool(name="work", bufs=2) as wp,
            tc.tile_pool(name="small", bufs=2) as sp,
            tc.tile_pool(name="ps", bufs=2, space="PSUM") as ps,
            tc.tile_pool(name="ps1", bufs=2, space="PSUM") as ps1,
            tc.tile_pool(name="pp", bufs=2, space="PSUM") as pp,
        ):
            # ---- image-0 x load first: it heads the critical path ----
            x_ts = [xp.tile([128, KT, T], F32, tag="x", name=f"x{i}") for i in range(BL)]
            for kt in range(KT):
                nc.sync.dma_start(out=x_ts[0][:, kt, :], in_=x_d[0, kt * 128:(kt + 1) * 128, :])

            # ---- x1 immediately after x0 on SP (keeps the hoisted img1
            # bn_stats from stalling the DVE FIFO); tiny consts first on
            # gpsimd, then weights in first-use order ----
            gated_insts = []
            for img in range(1, BL):
                for kt in range(KT):
                    di = nc.sync.dma_start(out=x_ts[img][:, kt, :],
                                           in_=x_d[img, kt * 128:(kt + 1) * 128, :])
                    gated_insts.append(di)
            ones_sb = cp.tile([128, 4, CH], F32R, tag="onesb")
            nc.sync.dma_start(out=ones_sb, in_=ones_d[:, :, :])
            r_sb = cp.tile([128, G], F32, tag="rm")
            nc.gpsimd.dma_start(out=r_sb, in_=r_d[:, :])
            bm_sb = cp.tile([G, 128], F32, tag="bm")
            nc.gpsimd.dma_start(out=bm_sb, in_=b_d[:, :])
            bq_sb = cp.tile([128, KT], F32, tag="bq")
            nc.gpsimd.dma_start(out=bq_sb, in_=bq_d[:, :])
            bk_sb = cp.tile([128, KT], F32, tag="bk")
            nc.gpsimd.dma_start(out=bk_sb, in_=bk_d[:, :])
            ob_sb = cp.tile([128, KT], F32, tag="ob")
            nc.gpsimd.dma_start(out=ob_sb, in_=ob_d[:, :])
            wq_sb = cp.tile([128, KT, C], F32R, tag="wq")
            wk_sb = cp.tile([128, KT, C], F32R, tag="wk")
            wv_sb = cp.tile([128, KT, C], F32R, tag="wv")
            wp_sb = cp.tile([128, KT, C], F32R, tag="wp")
            for w_sb, w_dr in ((wk_sb, wk_d), (wq_sb, wq_d), (wv_sb, wv_d), (wp_sb, wp_d)):
                nc.gpsimd.dma_start(
                    out=w_sb, in_=w_dr.rearrange("(kt p) m -> p kt m", p=128))

            # vT tiles: per s-block [t(128part), 4 pairs x (v_even|ones|v_odd)];
            # ones columns replicated from ones_sb by cheap DVE copies
            vt_all = [[dp.tile([128, 4, 3, CH], F32R, tag=f"vt{s}", bufs=2,
                                name=f"vt{img}_{s}") for s in range(8)]
                      for img in range(BL)]

            xn_ts = [None] * BL
            q_ts = [None] * BL
            k_ts = [None] * BL
            a_ts = [None] * BL

            def front(img):
                """x DMA (img>0), per-kt GroupNorm stats -> xn (each 128-ch
                tile holds exactly 8 whole groups, so the chain pipelines)."""
                x_t = x_ts[img]
                xn_t = dp.tile([128, KT, T], F32R, tag="xn", name=f"xn{img}")
                sts = []
                for kt in range(KT):
                    bn = sp.tile([128, 2, 6], F32, tag="bn")
                    nc.vector.bn_stats(out=bn[:, 0, :], in_=x_t[:, kt, 0:512])
                    nc.vector.bn_stats(out=bn[:, 1, :], in_=x_t[:, kt, 512:1024])
                    mv = sp.tile([128, 2], F32, tag="mv")
                    nc.vector.bn_aggr(out=mv, in_=bn)
                    st = sp.tile([128, 2], F32, tag=f"st{kt}", name=f"st{img}_{kt}")
                    nc.vector.tensor_copy(out=st[:, 0:1], in_=mv[:, 0:1])
                    nc.vector.tensor_mul(out=st[:, 1:2], in0=mv[:, 0:1], in1=mv[:, 0:1])
                    nc.vector.tensor_add(out=st[:, 1:2], in0=st[:, 1:2], in1=mv[:, 1:2])
                    sts.append(st)
                gsum = pp.tile([8, 2 * KT], F32, tag="pm", name=f"gsum{img}")
                for kt in range(KT):
                    nc.tensor.matmul(gsum[:, 2 * kt:2 * kt + 2], r_sb[:, 0:8], sts[kt],
                                     start=True, stop=True)
                gs_sb = sp.tile([8, 2 * KT], F32, tag="gs")
                nc.vector.tensor_copy(out=gs_sb, in_=gsum)
                gstat = sp.tile([8, 2 * KT], F32, tag="gstat")  # [rstd, beta] per kt
                var = sp.tile([8, KT], F32, tag="gvar")
                tmp = sp.tile([8, KT], F32, tag="gtmp")
                y = sp.tile([8, KT], F32, tag="gy")
                ge = gs_sb[:, 0:2 * KT:2]
                go = gs_sb[:, 1:2 * KT:2]
                nc.vector.tensor_mul(out=tmp, in0=ge, in1=ge)
                nc.vector.tensor_sub(out=var, in0=go, in1=tmp)
                nc.vector.tensor_scalar_add(out=var, in0=var, scalar1=EPS)
                # rsqrt via bit-trick seed + 3 Newton steps (all DVE; the
                # magic-constant subtraction runs in fp32 value domain since
                # DVE integer add is routed through fp32)
                u32 = mybir.dt.uint32
                yu = sp.tile([8, KT], u32, tag="gyu")
                nc.vector.tensor_scalar(out=yu, in0=var.bitcast(u32), scalar1=1,
                                        scalar2=None, op0=OP.logical_shift_right)
                nc.vector.tensor_copy(out=tmp, in_=yu)
                nc.vector.tensor_scalar(out=tmp, in0=tmp,
                                        scalar1=float(0x5f3759df), scalar2=-1.0,
                                        op0=OP.subtract, op1=OP.mult)
                nc.vector.tensor_copy(out=y.bitcast(u32), in_=tmp)
                for _ in range(3):
                    nc.vector.tensor_mul(out=tmp, in0=y, in1=y)
                    nc.vector.tensor_mul(out=tmp, in0=tmp, in1=var)
                    nc.vector.tensor_scalar(out=tmp, in0=tmp, scalar1=-0.5,
                                            scalar2=1.5, op0=OP.mult, op1=OP.add)
                    nc.vector.tensor_mul(out=y, in0=y, in1=tmp)
                nc.vector.tensor_copy(out=gstat[:, 0:2 * KT:2], in_=y)
                # beta = -mean * rstd
                nc.vector.tensor_mul(out=tmp, in0=ge, in1=y)
                nc.vector.tensor_scalar_mul(out=gstat[:, 1:2 * KT:2], in0=tmp,
                                            scalar1=-1.0)
                cstat = pp.tile([128, 2 * KT], F32, tag="pm", name=f"cstat{img}")
                for kt in range(KT):
                    nc.tensor.matmul(cstat[:, 2 * kt:2 * kt + 2], bm_sb[0:8, :],
                                     gstat[:, 2 * kt:2 * kt + 2],
                                     start=True, stop=True)
                cs_sb = sp.tile([128, 2 * KT], F32, tag="cs")
                nc.vector.tensor_copy(out=cs_sb, in_=cstat)
                for kt in range(KT):
                    nc.vector.tensor_scalar(out=xn_t[:, kt, :], in0=x_t[:, kt, :],
                                            scalar1=cs_sb[:, 2 * kt:2 * kt + 1],
                                            scalar2=cs_sb[:, 2 * kt + 1:2 * kt + 2],
                                            op0=OP.mult, op1=OP.add)
                for sblk in range(8):
                    ci = nc.vector.tensor_copy(out=vt_all[img][sblk][:, :, 1, :],
                                               in_=ones_sb)
                    if img > 0:
                        gated_insts.append(ci)
                xn_ts[img] = xn_t
                q_ts[img] = dp.tile([128, KT, T], F32R, tag="q", name=f"q{img}")
                k_ts[img] = dp.tile([128, KT, T], F32R, tag="k", name=f"k{img}")
                a_ts[img] = dp.tile([128, KT, T], F32R, tag="a", name=f"a{img}")

            def emit_v(img, s):
                pm = pp.tile([128, 512], F32, tag="pm", name=f"v{img}_{s}")
                for kt in range(KT):
                    mm(pm, xn_ts[img][:, kt, s * 128:(s + 1) * 128], wv_sb[:, kt, :],
                       kt == 0, kt == KT - 1)
                nc.vector.tensor_copy(
                    out=vt_all[img][s][:, :, 0:3:2, :],
                    in_=pm.rearrange("p (pr t c) -> p pr t c", pr=4, t=2))

            def emit_qk_group(img, which, mt, chk):
                w_sb, b_sb = (wq_sb, bq_sb) if which == "q" else (wk_sb, bk_sb)
                dst = q_ts[img] if which == "q" else k_ts[img]
                pm = pp.tile([128, 512], F32, tag="pm", name=f"qk{img}_{which}{mt}{chk}")
                for kt in range(KT):
                    mm(pm, w_sb[:, kt, mt * 128:(mt + 1) * 128],
                       xn_ts[img][:, kt, chk * 512:(chk + 1) * 512],
                       kt == 0, kt == KT - 1)
                nc.vector.tensor_scalar_add(
                    out=dst[:, mt, chk * 512:(chk + 1) * 512], in0=pm,
                    scalar1=b_sb[:, mt:mt + 1])

            def emit_proj(img, mt, chk, pool=None, ptag="pm"):
                pool = pool or pp
                pm = pool.tile([128, 512], F32, tag=ptag, name=f"pj{img}_{mt}{chk}")
                for kt in range(KT):
                    mm(pm, wp_sb[:, kt, mt * 128:(mt + 1) * 128],
                       a_ts[img][:, kt, chk * 512:(chk + 1) * 512],
                       kt == 0, kt == KT - 1)
                o_t = wp.tile([128, 512], F32, tag="o", bufs=4)
                nc.vector.scalar_tensor_tensor(
                    out=o_t, in0=pm, scalar=ob_sb[:, mt:mt + 1],
                    in1=x_ts[img][:, mt, chk * 512:(chk + 1) * 512],
                    op0=OP.add, op1=OP.add)
                eng = nc.sync if (mt + chk) % 2 == 0 else nc.gpsimd
                eng.dma_start(
                    out=y_d[img, mt * 128:(mt + 1) * 128, chk * 512:(chk + 1) * 512],
                    in_=o_t)

            # ---- prolog: image 0 front-end + v(s<2) + qk(mt=0, chk=0) ----
            front(0)
            emit_qk_group(0, "k", 0, 0)
            emit_qk_group(0, "k", 0, 1)
            emit_qk_group(0, "q", 0, 0)

            # ---- pending-work schedule keyed by global unit index ----
            sched = {}

            def put(u, fn):
                sched.setdefault(u, []).append(fn)

            for s in range(2, 8):
                put(s - 1, lambda s=s: emit_v(0, s))
            put(5, lambda: emit_qk_group(0, "q", 0, 1))
            # per-mt order: k chk0, q chk0, k chk1, q chk1
            qk4 = [("k", 0), ("q", 0), ("k", 1), ("q", 1)]
            for j, (w, chk) in enumerate(qk4):
                put(9 + j, lambda w=w, chk=chk: emit_qk_group(0, w, 1, chk))
                put(25 + j, lambda w=w, chk=chk: emit_qk_group(0, w, 2, chk))
                put(37 + j, lambda w=w, chk=chk: emit_qk_group(0, w, 3, chk))
            if BL > 1:
                put(44, lambda: front(1))
                for j, (w, chk) in enumerate(qk4):
                    put(56 + j, lambda w=w, chk=chk: emit_qk_group(1, w, 0, chk))
                    put(73 + j, lambda w=w, chk=chk: emit_qk_group(1, w, 1, chk))
                    put(89 + j, lambda w=w, chk=chk: emit_qk_group(1, w, 2, chk))
                    put(105 + j, lambda w=w, chk=chk: emit_qk_group(1, w, 3, chk))
                for s in range(8):
                    put(48 + s, lambda s=s: emit_v(1, s))
                for j in range(8):
                    put(65 + j, lambda j=j: emit_proj(0, j // 2, j % 2))

            # ---- attention: flat pipeline over (img, pair, t-half, s) units.
            # The two heads of a pair run as concurrent row-tiled QK matmuls
            # (lhsT partitions 0-63 vs 64-127); one exp covers both heads'
            # t-half chunk; PV accumulates into per-head 1-bank psum halves.
            units = [(img, pr, th, s) for img in range(BL)
                     for pr in range(4) for th in range(2) for s in range(8)]

            def emit_qkmm(img, pr, th, s):
                S = ps.tile([128, T], F32, tag="ps", name=f"S{img}_{pr}_{th}_{s}")
                for odd in range(2):
                    mm(S[:, odd * 512:(odd + 1) * 512],
                       k_ts[img][odd * 64:(odd + 1) * 64, pr, s * 128:(s + 1) * 128],
                       q_ts[img][odd * 64:(odd + 1) * 64, pr, th * 512:(th + 1) * 512],
                       True, True)
                return S

            gate_box = []
            pva = pvb = None
            p_prev = None
            S_cur = emit_qkmm(*units[0])
            for s_ in range(2):
                emit_v(0, s_)
            for i, (img, pr, th, s) in enumerate(units):
                p_t = wp.tile([128, T], F32R, tag="p", bufs=4)
                exp_inst = nc.scalar.activation(out=p_t, in_=S_cur, func=AF.Exp)
                if i == 2:
                    gate_box.append(exp_inst)
                if i + 1 < len(units):
                    S_cur = emit_qkmm(*units[i + 1])
                vflat = vt_all[img][s].rearrange("p pr t c -> p (pr t c)")
                if s == 0:
                    # defer PV(s=0) one unit: the previous pair's normalize
                    # still holds the pv slots
                    p_prev = p_t
                else:
                    if s == 1:
                        pva = ps1.tile([128, 512], F32, tag="pv",
                                       name=f"pva{img}_{pr}_{th}")
                        pvb = ps1.tile([128, 512], F32, tag="pv",
                                       name=f"pvb{img}_{pr}_{th}")
                        vflat0 = vt_all[img][0].rearrange("p pr t c -> p (pr t c)")
                        mm(pva, vflat0[:, pr * 192: pr * 192 + 128],
                           p_prev[:, 0:512], True, False)
                        mm(pvb, vflat0[:, pr * 192 + 64: pr * 192 + 192],
                           p_prev[:, 512:1024], True, False)
                    mm(pva, vflat[:, pr * 192: pr * 192 + 128],
                       p_t[:, 0:512], False, s == 7)
                    mm(pvb, vflat[:, pr * 192 + 64: pr * 192 + 192],
                       p_t[:, 512:1024], False, s == 7)
                for fn in sched.pop(i, ()):
                    fn()
                if s == 7:
                    rb = wp.tile([128, 512], F32, tag="rb")
                    tch = slice(th * 512, (th + 1) * 512)
                    nc.vector.reciprocal(out=rb[0:64, :], in_=pva[64:128, :])
                    nc.vector.tensor_mul(out=a_ts[img][0:64, pr, tch],
                                         in0=pva[0:64, :], in1=rb[0:64, :])
                    nc.vector.reciprocal(out=rb[64:128, :], in_=pvb[0:64, :])
                    nc.vector.tensor_mul(out=a_ts[img][64:128, pr, tch],
                                         in0=pvb[64:128, :], in1=rb[64:128, :])

            for gi in gated_insts:
                add_dep_helper(gi.ins, gate_box[0].ins, sync=False,
                               reason="hold img1 prefetch until attention starts")

            # ---- tail: proj of the last image, fanned across psum pools ----
            for j in range(8):
                mt, chk = j // 2, j % 2
                if j % 3 == 0:
                    emit_proj(BL - 1, mt, chk)
                elif j % 3 == 1:
                    emit_proj(BL - 1, mt, chk, pool=ps, ptag="ps")
                else:
                    emit_proj(BL - 1, mt, chk, pool=ps1, ptag="pv")
    nc.finalize()
    return nc


def get_nc():
    if "nc" not in _CACHE:
        _CACHE["nc"] = _build_nc()
    return _CACHE["nc"]


def prep_inputs(x, gn_w, gn_b, qkv_w, qkv_b, proj_w, proj_b):
    """Host-side preprocessing -> list of per-core input maps."""
    f = np.float32
    x = np.asarray(x, f)
    gn_w, gn_b = np.asarray(gn_w, f), np.asarray(gn_b, f)
    qkv_w, qkv_b = np.asarray(qkv_w, f), np.asarray(qkv_b, f)
    proj_w, proj_b = np.asarray(proj_w, f), np.asarray(proj_b, f)

    scale = 1.0 / np.sqrt(np.sqrt(CH))
    W3 = qkv_w.reshape(NH, 3 * CH, C)
    b3 = qkv_b.reshape(NH, 3 * CH)
    Wq = W3[:, 0:CH].reshape(C, C)
    Wk = W3[:, CH:2 * CH].reshape(C, C)
    Wv = W3[:, 2 * CH:3 * CH].reshape(C, C)
    bq = b3[:, 0:CH].reshape(C)
    bk = b3[:, CH:2 * CH].reshape(C)
    bv = b3[:, 2 * CH:3 * CH].reshape(C)

    wqT = ((Wq * gn_w[None, :]) * scale).T.copy()
    wkT = ((Wk * gn_w[None, :]) * scale).T.copy()
    wvT = (Wv * gn_w[None, :]).T.copy()
    bq_e = (Wq @ gn_b + bq) * scale
    bk_e = (Wk @ gn_b + bk) * scale
    bv_e = Wv @ gn_b + bv
    projT = proj_w.T.copy()
    ob = proj_w @ bv_e + proj_b

    gs = C // G  # 16 channels per group; 8 whole groups per 128-channel tile
    Rm = np.zeros((128, G), f)
    Rm[np.arange(128), np.arange(128) // gs] = 1.0 / gs
    Bm = np.zeros((G, 128), f)
    Bm[np.arange(128) // gs, np.arange(128)] = 1.0

    def pack_bias(b):
        return np.ascontiguousarray(b.reshape(KT, 128).T)

    x5 = x.reshape(B, C, T)
    shared = dict(wqT=wqT, wkT=wkT, wvT=wvT, projT=projT, Rm=Rm, Bm=Bm,
                  bq=pack_bias(bq_e), bk=pack_bias(bk_e), ob=pack_bias(ob),
                  ones=np.ones((128, 4, CH), f))
    return [dict(x=np.ascontiguousarray(x5[c * BL:(c + 1) * BL]), **shared)
            for c in range(NCORES)]


def kernel(x, gn_w, gn_b, qkv_w, qkv_b, proj_w, proj_b):
    from concourse.bass_utils import run_bass_kernel_spmd

    in_maps = prep_inputs(x, gn_w, gn_b, qkv_w, qkv_b, proj_w, proj_b)
    nc = get_nc()
    try:
        res = run_bass_kernel_spmd(nc, in_maps, list(range(NCORES)))
    except Exception:
        # transient NRT device wedges have been observed to clear on retry
        time.sleep(2.0)
        res = run_bass_kernel_spmd(nc, in_maps, list(range(NCORES)))
    out = np.concatenate([res.results[c]["y"] for c in range(NCORES)], axis=0)
    return out.reshape(B, C, HW, HW).astype(np.float32)
